# revision 3
# baseline (speedup 1.0000x reference)
"""Involution-style aggregation (nn_AggregationNonCupy) on 8 Trainium2 NeuronCores.

out[n, g*32+cw, y, x] = sum_{di,dj in {-1,0,1}} weight[n, cw, (di+1)*3+dj+1, y*64+x]
                        * input[n, g*32+cw, y+di, x+dj]      (zero padded)

v3 design, ~244us HW (baseline v2: ~447us). Built from HW microbenchmarks:
  * DMA is the wall. Uniform-stride 2-dim-AP transfers reach ~330-370 GB/s
    per core; 3-dim-AP transfers crawl at ~100 GB/s regardless of descriptor
    size. So every DRAM transfer here is a 2-dim uniform-stride instruction,
    with the host pre-permuting input/output to make per-group slices fully
    contiguous (2MB per instruction).
  * Pool SWDGE casting DMAs (fp32->fp16 during the transfer) are bit-exact
    round-to-nearest and run at full rate: the input is cast on the fly, so
    there is no fp32 staging and no ACT conversion for it.
  * DVE fp16 2x tensor_tensor mode does NOT require 4B-aligned APs on this
    HW (measured identical rate and exact results at odd-element offsets),
    so the dj=+-1 taps read the input block directly - no shifted copy.
  * Pool's tensor_mul measured ~5-10x slower than the cost model; any Pool
    product share also serialized DVE via shared pool-buffer reuse. All 144
    product chunks run on DVE (~1.13us per [128,2048] chunk).

Structure:
  - Sharding: core = (batch-quad bq, group-half gh): batches 4bq..4bq+3,
    channels 256gh..256gh+255 (8 groups of 32 = one weight-group half).
    Weight slice [4 batches] is read by both gh cores of a quad.
  - Partition dim = (n4, cw) = 128. Per group each partition holds the FULL
    64x64 channel map: vertical taps are free-dim offsets (no halo reads),
    x-boundary taps use column-zeroed fp16 weights.
  - Per body: 9 weight tap-chunks stream on the SP HWDGE queue (fp32) with
    ACT converting into a resident [128, 36864] fp16 tile; 8 input casting
    DMAs stream on the SWDGE queue into [128, 4228] padded fp16 blocks;
    per (group, half): 9 DVE products -> PE identity-matmul accumulation
    into PSUM fp32 -> ACT evacuation -> 2MB contiguous store on SP.
"""

import os

import numpy as np

import concourse.bacc as bacc
import concourse.mybir as mybir
import concourse.tile as tile
from concourse.bass_utils import run_bass_kernel_spmd

# Problem constants (hardcoded per harness contract)
N_TOTAL, C_X, H, W = 16, 512, 64, 64
C_W = 32
N_CORES = 8
N4 = 4             # batches per core
G8 = 8             # groups per core
HW_ = H * W        # 4096
IM = 2 + W         # image offset in the block (4B-aligned for the cast DMA)
BLK = IM + HW_ + W + 2  # 4228: [z2, pad row, image, pad row, z2]
WCOLS = 9 * HW_    # 36864 weight elems per partition
MM_N = 512         # matmul chunk (one PSUM bank of fp32)
HALF = HW_ // 2    # 2048

# tap list: k = (di+1)*3 + (dj+1)
TAPS = [(di, dj) for di in (-1, 0, 1) for dj in (-1, 0, 1)]

# All products run on DVE: Pool's tensor_mul measured ~10us per [128,2048]
# chunk on HW (vs DVE 1.13us at 2x), and any Pool share serialized the flow.
POOL_BY_GROUP = [set() for _ in range(8)]
PIPE = 3           # input-stage lookahead (groups)
# debug flag set (comma-separated): nodma, nocompute, nomm, noevac, alldve
MODE = {"full"}


def emit_kernel(tc, x, wgt, o, reps=1):
    nc = tc.nc
    f32 = mybir.dt.float32
    f16 = mybir.dt.float16

    # x/o arrive host-permuted as [g, (n cw), l]: per-group transfers are a
    # single fully-contiguous 2MB uniform-stride instruction (~366 GB/s HW
    # path; 3-dim APs only reach ~100 GB/s)
    xv = x
    ov = o
    wv = wgt.rearrange("n cw k l -> (n cw) (k l)")        # [128, 36864]

    ident_dram = nc.inline_tensor(np.eye(128, dtype=np.float16), name="ident")

    with (
        tc.tile_pool(name="const", bufs=1) as const_pool,
        tc.tile_pool(name="w16", bufs=1) as w16_pool,
        tc.tile_pool(name="ina", bufs=4) as ina_pool,
        tc.tile_pool(name="prod", bufs=6) as prod_pool,
        tc.tile_pool(name="psum", bufs=2, space="PSUM") as psum_pool,
        tc.tile_pool(name="wst", bufs=2) as wst_pool,
        tc.tile_pool(name="outp", bufs=2) as out_pool,
    ):
        ident = const_pool.tile([128, 128], f16)
        nc.sync.dma_start(ident[:], ident_dram.ap())

        env = dict(locals())

        if reps == 1:
            _emit_body(tc, env)
        else:
            with tc.For_i(0, reps, 1):
                _emit_body(tc, env)


def _emit_body(tc, env):
    nc = env["nc"]
    f32, f16 = env["f32"], env["f16"]
    xv, ov, wv, ident = env["xv"], env["ov"], env["wv"], env["ident"]
    w16_pool = env["w16_pool"]
    ina_pool = env["ina_pool"]
    prod_pool, psum_pool, out_pool = (env["prod_pool"], env["psum_pool"],
                                      env["out_pool"])
    wst_pool = env["wst_pool"]

    # ---- weights: fp16 resident tile [128, 36864], SWDGE casting loads
    wt16 = w16_pool.tile([128, WCOLS], f16, tag="wt16")

    def load_weights():
        # fp32 loads on the SP HWDGE queue (idle at body start; the SWDGE
        # queue is reserved for input casts) + ACT fp32->fp16 conversion.
        # Per-tap chunks so the first products unblock after one chunk.
        wview = wt16.rearrange("p (k y xx) -> p k y xx", k=9, xx=W)
        for k, (di, dj) in enumerate(TAPS):
            if "nodma" not in MODE:
                ws = wst_pool.tile([128, HW_], f32, tag="ws")
                nc.sync.dma_start(ws[:], wv[:, k * HW_:(k + 1) * HW_])
                if "nocompute" in MODE:
                    continue
                nc.scalar.copy(wt16[:, k * HW_:(k + 1) * HW_], ws[:])
            if "nocompute" in MODE:
                continue
            # zero weight columns at x-boundaries: dj=-1 taps kill x=0,
            # dj=+1 taps kill x=63 (their input reads are out-of-range wraps)
            if dj != 0:
                col = 0 if dj == -1 else W - 1
                nc.gpsimd.memset(wview[:, k, :, col:col + 1], 0.0)

    stage = {}

    def input_stage(g):
        ita = ina_pool.tile([128, BLK], f16, tag="ita")
        nc.gpsimd.memset(ita[:, 0:IM], 0.0)
        nc.gpsimd.memset(ita[:, IM + HW_:BLK], 0.0)
        # fp32->fp16 casting DMA (SWDGE), fully-contiguous 2MB source
        if "nodma" not in MODE:
            nc.gpsimd.dma_start(ita[:, IM:IM + HW_], xv[g])
        stage[g] = ita

    def compute_stage(g):
        ita = stage.pop(g)
        pool_chunks = POOL_BY_GROUP[g]
        ot = out_pool.tile([128, HW_], f32, tag="ot")
        if "nocompute" in MODE:
            if "nodma" not in MODE:
                nc.sync.dma_start(ov[g], ot[:])
            return
        use_pool = "alldve" not in MODE

        def src_for(k, half):
            # odd-elem offsets verified to keep DVE 2x mode on HW, so the
            # dj=+-1 taps read ita directly (no shifted copy needed).
            # Edge reads land in the zeroed pad rows / lead+tail elems.
            di, dj = TAPS[k]
            s = IM + W * di + dj + half * HALF
            return ita[:, s:s + HALF]

        def wk_for(k, half):
            return wt16[:, k * HW_ + half * HALF:k * HW_ + (half + 1) * HALF]

        pool_pk = {}

        for half in range(2):
            ps = psum_pool.tile([128, HALF], f32, tag="ps")
            # DVE taps first (available soonest), Pool taps consumed last —
            # the tap sum is order-independent, so this gives Pool products
            # maximum slack before PE needs them
            order = [(k, half) for k in range(9)
                     if not (use_pool and (k, half) in pool_chunks)]
            order += sorted(kh for kh in pool_chunks
                            if kh[1] == half and use_pool)
            for i, (k, half_) in enumerate(order):
                if (k, half_) in pool_pk:
                    pk = pool_pk[(k, half_)]
                else:
                    pk = prod_pool.tile([128, HALF], f16, tag="pk")
                    nc.vector.tensor_mul(pk[:], wk_for(k, half_),
                                         src_for(k, half_))
                if "nomm" in MODE:
                    continue
                for c in range(0, HALF, MM_N):
                    nc.tensor.matmul(ps[:, c:c + MM_N], ident[:],
                                     pk[:, c:c + MM_N],
                                     start=(i == 0), stop=(i == len(order) - 1))
            if "nomm" in MODE or "noevac" in MODE:
                continue
            nc.scalar.copy(ot[:, half * HALF:(half + 1) * HALF], ps[:])
        # fully-contiguous 2MB store (host un-permutes); SP HWDGE queue is
        # otherwise idle (input+weights ride the Pool SWDGE queue)
        if not MODE & {"nodma", "nomm", "noevac"}:
            nc.sync.dma_start(ov[g], ot[:])

    # startup: interleave the weight-cast chunks with the first input loads
    # on the shared SWDGE queue so the first products start after
    # in0 + w[taps 0-2] rather than after the whole weight phase
    input_stage(0)
    load_weights()
    for g in range(1, min(PIPE, G8)):
        input_stage(g)
    for g in range(G8):
        if g + PIPE < G8:
            input_stage(g + PIPE)
        compute_stage(g)


def build_program(reps=1):
    nc = bacc.Bacc("TRN2", target_bir_lowering=False, debug=False,
                   enable_asserts=True, num_devices=N_CORES)
    f32 = mybir.dt.float32
    x = nc.dram_tensor("x", [G8, 128, HW_], f32, kind="ExternalInput").ap()
    wgt = nc.dram_tensor("w", [N4, C_W, 9, HW_], f32,
                         kind="ExternalInput").ap()
    o = nc.dram_tensor("o", [G8, 128, HW_], f32, kind="ExternalOutput").ap()
    with tile.TileContext(nc) as tc:
        emit_kernel(tc, x, wgt, o, reps=reps)
    nc.compile()
    return nc


_CACHED_NC = None


def _get_nc():
    global _CACHED_NC
    if _CACHED_NC is None:
        _CACHED_NC = build_program()
    return _CACHED_NC


def run(inputs, trace=False):
    """Run on 8 cores; returns (output [16,512,64,64] fp32, BassKernelResults)."""
    inp = np.ascontiguousarray(np.asarray(inputs["input"], dtype=np.float32))
    wgt = np.ascontiguousarray(np.asarray(inputs["weight"], dtype=np.float32))
    assert inp.shape == (N_TOTAL, C_X, H, W)
    assert wgt.shape == (N_TOTAL, C_W, 9, HW_)

    nc = _get_nc()
    inp = inp.reshape(N_TOTAL, C_X, HW_)
    in_maps = []
    for c in range(N_CORES):
        bq, gh = divmod(c, 2)
        nsl = slice(4 * bq, 4 * bq + 4)
        csl = slice(256 * gh, 256 * gh + 256)
        # host-permute input to [g, n, cw, l] so device transfers are
        # fully contiguous per group
        xc = inp[nsl, csl].reshape(N4, G8, C_W, HW_).transpose(1, 0, 2, 3)
        in_maps.append({
            "x": np.ascontiguousarray(xc).reshape(G8, 128, HW_),
            "w": np.ascontiguousarray(wgt[nsl]),
        })
    res = run_bass_kernel_spmd(nc, in_maps, core_ids=list(range(N_CORES)),
                               trace=trace)
    out = np.empty((N_TOTAL, C_X, HW_), dtype=np.float32)
    for c in range(N_CORES):
        bq, gh = divmod(c, 2)
        oc = res.results[c]["o"].reshape(G8, N4, C_W, HW_).transpose(1, 0, 2, 3)
        out[4 * bq:4 * bq + 4, 256 * gh:256 * gh + 256] = \
            oc.reshape(N4, 256, HW_)
    return out.reshape(N_TOTAL, C_X, H, W), res


def kernel(**inputs):
    out, _ = run(inputs)
    return out


# revision 4
# speedup vs baseline: 1.1175x; 1.1175x over previous
"""Involution-style aggregation — v4: spatial-half sharding (see kernel.py v3).

Core = (batch-quad bq, y-half yh): batches 4bq..4bq+4, output rows
32yh..32yh+32, ALL 512 channels (16 groups). vs v3's channel-half sharding
this halves per-core weight traffic (9.4MB vs 18.9MB — each weight element
is read once fleet-wide) and shrinks the body-start weight ramp.

Host bakes halo rows and zero padding into the permuted input (blocks of
[z2, 34 rows x 64, z2] = 2180 fp16 elems, two groups packed per 17.4KB-desc
casting DMA), so the device does no pad memsets at all. Weights stream as
3-tap fp32 chunks on SP + ACT conversion into a resident [128, 18432] fp16
tile. Products all on DVE (2x fp16, unaligned APs fine); PE identity-matmul
tap accumulation into PSUM; ACT evacuates; 2-group 16KB-desc stores on SP.
"""

import numpy as np

import concourse.bacc as bacc
import concourse.mybir as mybir
import concourse.tile as tile
from concourse.bass_utils import run_bass_kernel_spmd

# Problem constants (hardcoded per harness contract)
N_TOTAL, C_X, H, W = 16, 512, 64, 64
C_W = 32
N_CORES = 8
N4 = 4              # batches per core
G16 = 16            # groups per core
ROWS = 34           # 32 output rows + 2 halo rows per block
GBLK = 2 + ROWS * W + 2   # 2180: [z2, 34 rows, z2]
PBLK = 2 * GBLK     # 4360: two groups per input DMA
GOUT = 32 * W       # 2048 output elems per group
WCOLS = 9 * GOUT    # 18432 weight elems per partition
WCH = 3 * GOUT      # 6144: 3-tap weight chunk
MM_N = 512
PIPE = 2            # pair lookahead

TAPS = [(di, dj) for di in (-1, 0, 1) for dj in (-1, 0, 1)]


def emit_kernel(tc, x, wgt, o, reps=1):
    nc = tc.nc
    f32 = mybir.dt.float32
    f16 = mybir.dt.float16

    ident_dram = nc.inline_tensor(np.eye(128, dtype=np.float16), name="ident")

    with (
        tc.tile_pool(name="const", bufs=1) as const_pool,
        tc.tile_pool(name="w16", bufs=1) as w16_pool,
        tc.tile_pool(name="ina", bufs=4) as ina_pool,
        tc.tile_pool(name="prod", bufs=6) as prod_pool,
        tc.tile_pool(name="psum", bufs=2, space="PSUM") as psum_pool,
        tc.tile_pool(name="wst", bufs=2) as wst_pool,
        tc.tile_pool(name="outp", bufs=2) as out_pool,
    ):
        ident = const_pool.tile([128, 128], f16)
        nc.sync.dma_start(ident[:], ident_dram.ap())
        env = dict(locals())
        if reps == 1:
            _emit_body(tc, env)
        else:
            with tc.For_i(0, reps, 1):
                _emit_body(tc, env)


def _emit_body(tc, env):
    nc = env["nc"]
    f32, f16 = env["f32"], env["f16"]
    x, wgt, o, ident = env["x"], env["wgt"], env["o"], env["ident"]
    w16_pool, ina_pool = env["w16_pool"], env["ina_pool"]
    prod_pool, psum_pool = env["prod_pool"], env["psum_pool"]
    wst_pool, out_pool = env["wst_pool"], env["out_pool"]

    wt16 = w16_pool.tile([128, WCOLS], f16, tag="wt16")

    def load_weights():
        wview = wt16.rearrange("p (k y xx) -> p k y xx", k=9, xx=W)
        for kb in range(3):
            ws = wst_pool.tile([128, WCH], f32, tag="ws")
            nc.sync.dma_start(ws[:], wgt[kb])
            nc.scalar.copy(wt16[:, kb * WCH:(kb + 1) * WCH], ws[:])
            for dk in range(3):
                k = kb * 3 + dk
                di, dj = TAPS[k]
                if dj != 0:
                    col = 0 if dj == -1 else W - 1
                    nc.gpsimd.memset(wview[:, k, :, col:col + 1], 0.0)

    stage = {}

    def input_stage(p):
        ita = ina_pool.tile([128, PBLK], f16, tag="ita")
        # fp32->fp16 casting DMA; halos and zero pads are host-baked
        nc.gpsimd.dma_start(ita[:], x[p])
        stage[p] = ita

    def compute_stage(p):
        ita = stage.pop(p)
        ot = out_pool.tile([128, 2 * GOUT], f32, tag="ot")
        for slot in range(2):
            boff = slot * GBLK
            ps = psum_pool.tile([128, GOUT], f32, tag="ps")
            for k, (di, dj) in enumerate(TAPS):
                pk = prod_pool.tile([128, GOUT], f16, tag="pk")
                s = boff + 2 + (1 + di) * W + dj
                nc.vector.tensor_mul(pk[:],
                                     wt16[:, k * GOUT:(k + 1) * GOUT],
                                     ita[:, s:s + GOUT])
                for c in range(0, GOUT, MM_N):
                    nc.tensor.matmul(ps[:, c:c + MM_N], ident[:],
                                     pk[:, c:c + MM_N],
                                     start=(k == 0), stop=(k == 8))
            nc.scalar.copy(ot[:, slot * GOUT:(slot + 1) * GOUT], ps[:])
        nc.sync.dma_start(o[p], ot[:])

    input_stage(0)
    load_weights()
    for p in range(1, PIPE):
        input_stage(p)
    for p in range(8):
        if p + PIPE < 8:
            input_stage(p + PIPE)
        compute_stage(p)


def build_program(reps=1):
    nc = bacc.Bacc("TRN2", target_bir_lowering=False, debug=False,
                   enable_asserts=True, num_devices=N_CORES)
    f32 = mybir.dt.float32
    x = nc.dram_tensor("x", [8, 128, PBLK], f32, kind="ExternalInput").ap()
    wgt = nc.dram_tensor("w", [3, 128, WCH], f32, kind="ExternalInput").ap()
    o = nc.dram_tensor("o", [8, 128, 2 * GOUT], f32,
                       kind="ExternalOutput").ap()
    with tile.TileContext(nc) as tc:
        emit_kernel(tc, x, wgt, o, reps=reps)
    nc.compile()
    return nc


_CACHED_NC = None


def _get_nc():
    global _CACHED_NC
    if _CACHED_NC is None:
        _CACHED_NC = build_program()
    return _CACHED_NC


def run(inputs, trace=False):
    inp = np.ascontiguousarray(np.asarray(inputs["input"], dtype=np.float32))
    wgt = np.ascontiguousarray(np.asarray(inputs["weight"], dtype=np.float32))
    assert inp.shape == (N_TOTAL, C_X, H, W)
    assert wgt.shape == (N_TOTAL, C_W, 9, H * W)

    nc = _get_nc()
    inp5 = inp.reshape(N_TOTAL, G16, C_W, H, W)
    in_maps = []
    for c in range(N_CORES):
        bq, yh = divmod(c, 2)
        nsl = slice(4 * bq, 4 * bq + 4)
        # input blocks: [g, n, cw, 2180] with halo rows & zeros baked in
        blk = np.zeros((G16, N4, C_W, GBLK), np.float32)
        r0 = 32 * yh - 1
        lo, hi = max(r0, 0), min(r0 + ROWS, H)
        pad = lo - r0
        blk[..., 2 + pad * W:2 + (pad + hi - lo) * W] = \
            inp5[nsl, :, :, lo:hi].transpose(1, 0, 2, 3, 4).reshape(
                G16, N4, C_W, (hi - lo) * W)
        xc = blk.reshape(8, 2, N4, C_W, GBLK).transpose(0, 2, 3, 1, 4)
        # weights: y-half slice, 3-tap chunks [3, 128, 6144]
        wh = wgt[nsl].reshape(N4, C_W, 9, H * W)[
            :, :, :, yh * GOUT:(yh + 1) * GOUT]
        wc = wh.transpose(2, 0, 1, 3).reshape(3, 3, 128, GOUT) \
            .transpose(0, 2, 1, 3)
        in_maps.append({
            "x": np.ascontiguousarray(xc).reshape(8, 128, PBLK),
            "w": np.ascontiguousarray(wc).reshape(3, 128, WCH),
        })
    res = run_bass_kernel_spmd(nc, in_maps, core_ids=list(range(N_CORES)),
                               trace=trace)
    out = np.empty((N_TOTAL, C_X, H * W), dtype=np.float32)
    for c in range(N_CORES):
        bq, yh = divmod(c, 2)
        oc = res.results[c]["o"].reshape(8, N4, C_W, 2, GOUT) \
            .transpose(1, 0, 3, 2, 4).reshape(N4, C_X, GOUT)
        out[4 * bq:4 * bq + 4, :, yh * GOUT:(yh + 1) * GOUT] = oc
    return out.reshape(N_TOTAL, C_X, H, W), res


def kernel(**inputs):
    out, _ = run(inputs)
    return out


# revision 5
# speedup vs baseline: 1.1224x; 1.0043x over previous
"""Involution-style aggregation — v4: spatial-half sharding (see kernel.py v3).

Core = (batch-quad bq, y-half yh): batches 4bq..4bq+4, output rows
32yh..32yh+32, ALL 512 channels (16 groups). vs v3's channel-half sharding
this halves per-core weight traffic (9.4MB vs 18.9MB — each weight element
is read once fleet-wide) and shrinks the body-start weight ramp.

Host bakes halo rows and zero padding into the permuted input (blocks of
[z2, 34 rows x 64, z2] = 2180 fp16 elems, two groups packed per 17.4KB-desc
casting DMA), so the device does no pad memsets at all. Weights stream as
3-tap fp32 chunks on SP + ACT conversion into a resident [128, 18432] fp16
tile. Products all on DVE (2x fp16, unaligned APs fine); PE identity-matmul
tap accumulation into PSUM; ACT evacuates; 2-group 16KB-desc stores on SP.
"""

import numpy as np

import concourse.bacc as bacc
import concourse.mybir as mybir
import concourse.tile as tile
from concourse.bass_utils import run_bass_kernel_spmd

# Problem constants (hardcoded per harness contract)
N_TOTAL, C_X, H, W = 16, 512, 64, 64
C_W = 32
N_CORES = 8
N4 = 4              # batches per core
G16 = 16            # groups per core
ROWS = 34           # 32 output rows + 2 halo rows per block
GBLK = 2 + ROWS * W + 2   # 2180: [z2, 34 rows, z2]
PBLK = 2 * GBLK     # 4360: two groups per input DMA
GOUT = 32 * W       # 2048 output elems per group
WCOLS = 9 * GOUT    # 18432 weight elems per partition
WCH = 3 * GOUT      # 6144: 3-tap weight chunk
MM_N = 512
PIPE = 3            # pair lookahead

TAPS = [(di, dj) for di in (-1, 0, 1) for dj in (-1, 0, 1)]


def emit_kernel(tc, x, wgt, o, reps=1):
    nc = tc.nc
    f32 = mybir.dt.float32
    f16 = mybir.dt.float16

    ident_dram = nc.inline_tensor(np.eye(128, dtype=np.float16), name="ident")

    with (
        tc.tile_pool(name="const", bufs=1) as const_pool,
        tc.tile_pool(name="w16", bufs=1) as w16_pool,
        tc.tile_pool(name="ina", bufs=4) as ina_pool,
        tc.tile_pool(name="prod", bufs=8) as prod_pool,
        tc.tile_pool(name="psum", bufs=2, space="PSUM") as psum_pool,
        tc.tile_pool(name="wst", bufs=2) as wst_pool,
        tc.tile_pool(name="outp", bufs=2) as out_pool,
    ):
        ident = const_pool.tile([128, 128], f16)
        nc.sync.dma_start(ident[:], ident_dram.ap())
        env = dict(locals())
        if reps == 1:
            _emit_body(tc, env)
        else:
            with tc.For_i(0, reps, 1):
                _emit_body(tc, env)


def _emit_body(tc, env):
    nc = env["nc"]
    f32, f16 = env["f32"], env["f16"]
    x, wgt, o, ident = env["x"], env["wgt"], env["o"], env["ident"]
    w16_pool, ina_pool = env["w16_pool"], env["ina_pool"]
    prod_pool, psum_pool = env["prod_pool"], env["psum_pool"]
    wst_pool, out_pool = env["wst_pool"], env["out_pool"]

    wt16 = w16_pool.tile([128, WCOLS], f16, tag="wt16")

    def load_weights():
        wview = wt16.rearrange("p (k y xx) -> p k y xx", k=9, xx=W)
        for kb in range(3):
            ws = wst_pool.tile([128, WCH], f32, tag="ws")
            nc.sync.dma_start(ws[:], wgt[kb])
            nc.scalar.copy(wt16[:, kb * WCH:(kb + 1) * WCH], ws[:])
            for dk in range(3):
                k = kb * 3 + dk
                di, dj = TAPS[k]
                if dj != 0:
                    col = 0 if dj == -1 else W - 1
                    nc.gpsimd.memset(wview[:, k, :, col:col + 1], 0.0)

    stage = {}

    def input_stage(p):
        ita = ina_pool.tile([128, PBLK], f16, tag="ita")
        # fp32->fp16 casting DMA; halos and zero pads are host-baked
        nc.gpsimd.dma_start(ita[:], x[p])
        stage[p] = ita

    def compute_stage(p):
        ita = stage.pop(p)
        ot = out_pool.tile([128, 2 * GOUT], f32, tag="ot")
        for slot in range(2):
            boff = slot * GBLK
            ps = psum_pool.tile([128, GOUT], f32, tag="ps")
            for k, (di, dj) in enumerate(TAPS):
                pk = prod_pool.tile([128, GOUT], f16, tag="pk")
                s = boff + 2 + (1 + di) * W + dj
                nc.vector.tensor_mul(pk[:],
                                     wt16[:, k * GOUT:(k + 1) * GOUT],
                                     ita[:, s:s + GOUT])
                for c in range(0, GOUT, MM_N):
                    nc.tensor.matmul(ps[:, c:c + MM_N], ident[:],
                                     pk[:, c:c + MM_N],
                                     start=(k == 0), stop=(k == 8))
            nc.scalar.copy(ot[:, slot * GOUT:(slot + 1) * GOUT], ps[:])
        nc.scalar.dma_start(o[p], ot[:])

    input_stage(0)
    load_weights()
    for p in range(1, PIPE):
        input_stage(p)
    for p in range(8):
        if p + PIPE < 8:
            input_stage(p + PIPE)
        compute_stage(p)


def build_program(reps=1):
    nc = bacc.Bacc("TRN2", target_bir_lowering=False, debug=False,
                   enable_asserts=True, num_devices=N_CORES)
    f32 = mybir.dt.float32
    x = nc.dram_tensor("x", [8, 128, PBLK], f32, kind="ExternalInput").ap()
    wgt = nc.dram_tensor("w", [3, 128, WCH], f32, kind="ExternalInput").ap()
    o = nc.dram_tensor("o", [8, 128, 2 * GOUT], f32,
                       kind="ExternalOutput").ap()
    with tile.TileContext(nc) as tc:
        emit_kernel(tc, x, wgt, o, reps=reps)
    nc.compile()
    return nc


_CACHED_NC = None


def _get_nc():
    global _CACHED_NC
    if _CACHED_NC is None:
        _CACHED_NC = build_program()
    return _CACHED_NC


def run(inputs, trace=False):
    inp = np.ascontiguousarray(np.asarray(inputs["input"], dtype=np.float32))
    wgt = np.ascontiguousarray(np.asarray(inputs["weight"], dtype=np.float32))
    assert inp.shape == (N_TOTAL, C_X, H, W)
    assert wgt.shape == (N_TOTAL, C_W, 9, H * W)

    nc = _get_nc()
    inp5 = inp.reshape(N_TOTAL, G16, C_W, H, W)
    in_maps = []
    for c in range(N_CORES):
        bq, yh = divmod(c, 2)
        nsl = slice(4 * bq, 4 * bq + 4)
        # input blocks: [g, n, cw, 2180] with halo rows & zeros baked in
        blk = np.zeros((G16, N4, C_W, GBLK), np.float32)
        r0 = 32 * yh - 1
        lo, hi = max(r0, 0), min(r0 + ROWS, H)
        pad = lo - r0
        blk[..., 2 + pad * W:2 + (pad + hi - lo) * W] = \
            inp5[nsl, :, :, lo:hi].transpose(1, 0, 2, 3, 4).reshape(
                G16, N4, C_W, (hi - lo) * W)
        xc = blk.reshape(8, 2, N4, C_W, GBLK).transpose(0, 2, 3, 1, 4)
        # weights: y-half slice, 3-tap chunks [3, 128, 6144]
        wh = wgt[nsl].reshape(N4, C_W, 9, H * W)[
            :, :, :, yh * GOUT:(yh + 1) * GOUT]
        wc = wh.transpose(2, 0, 1, 3).reshape(3, 3, 128, GOUT) \
            .transpose(0, 2, 1, 3)
        in_maps.append({
            "x": np.ascontiguousarray(xc).reshape(8, 128, PBLK),
            "w": np.ascontiguousarray(wc).reshape(3, 128, WCH),
        })
    res = run_bass_kernel_spmd(nc, in_maps, core_ids=list(range(N_CORES)),
                               trace=trace)
    out = np.empty((N_TOTAL, C_X, H * W), dtype=np.float32)
    for c in range(N_CORES):
        bq, yh = divmod(c, 2)
        oc = res.results[c]["o"].reshape(8, N4, C_W, 2, GOUT) \
            .transpose(1, 0, 3, 2, 4).reshape(N4, C_X, GOUT)
        out[4 * bq:4 * bq + 4, :, yh * GOUT:(yh + 1) * GOUT] = oc
    return out.reshape(N_TOTAL, C_X, H, W), res


def kernel(**inputs):
    out, _ = run(inputs)
    return out


# revision 7
# speedup vs baseline: 1.1355x; 1.0117x over previous
"""Involution-style aggregation — v4: spatial-half sharding (see kernel.py v3).

Core = (batch-quad bq, y-half yh): batches 4bq..4bq+4, output rows
32yh..32yh+32, ALL 512 channels (16 groups). vs v3's channel-half sharding
this halves per-core weight traffic (9.4MB vs 18.9MB — each weight element
is read once fleet-wide) and shrinks the body-start weight ramp.

Host bakes halo rows and zero padding into the permuted input (blocks of
[z2, 34 rows x 64, z2] = 2180 fp16 elems, two groups packed per 17.4KB-desc
casting DMA), so the device does no pad memsets at all. Weights stream as
3-tap fp32 chunks on SP + ACT conversion into a resident [128, 18432] fp16
tile. Products all on DVE (2x fp16, unaligned APs fine); PE identity-matmul
tap accumulation into PSUM; ACT evacuates; 2-group 16KB-desc stores on SP.
"""

import numpy as np

import concourse.bacc as bacc
import concourse.mybir as mybir
import concourse.tile as tile
from concourse.bass_utils import run_bass_kernel_spmd

# Problem constants (hardcoded per harness contract)
N_TOTAL, C_X, H, W = 16, 512, 64, 64
C_W = 32
N_CORES = 8
N4 = 4              # batches per core
G16 = 16            # groups per core
ROWS = 34           # 32 output rows + 2 halo rows per block
GBLK = 2 + ROWS * W + 2   # 2180: [z2, 34 rows, z2]
PBLK = 2 * GBLK     # 4360: two groups per input DMA
GOUT = 32 * W       # 2048 output elems per group
WCOLS = 9 * GOUT    # 18432 weight elems per partition
WCH = 3 * GOUT      # 6144: 3-tap weight chunk
MM_N = 512
PIPE = 3
MODE = {"full"}            # pair lookahead

TAPS = [(di, dj) for di in (-1, 0, 1) for dj in (-1, 0, 1)]


def emit_kernel(tc, x, wgt, o, reps=1):
    nc = tc.nc
    f32 = mybir.dt.float32
    f16 = mybir.dt.float16

    ident_dram = nc.inline_tensor(np.eye(128, dtype=np.float16), name="ident")

    with (
        tc.tile_pool(name="const", bufs=1) as const_pool,
        tc.tile_pool(name="w16", bufs=1) as w16_pool,
        tc.tile_pool(name="ina", bufs=4) as ina_pool,
        tc.tile_pool(name="prod", bufs=8) as prod_pool,
        tc.tile_pool(name="psum", bufs=2, space="PSUM") as psum_pool,
        tc.tile_pool(name="wst", bufs=2) as wst_pool,
        tc.tile_pool(name="outp", bufs=2) as out_pool,
    ):
        ident = const_pool.tile([128, 128], f16)
        nc.sync.dma_start(ident[:], ident_dram.ap())
        env = dict(locals())
        if reps == 1:
            _emit_body(tc, env)
        else:
            with tc.For_i(0, reps, 1):
                _emit_body(tc, env)


def _emit_body(tc, env):
    nc = env["nc"]
    f32, f16 = env["f32"], env["f16"]
    x, wgt, o, ident = env["x"], env["wgt"], env["o"], env["ident"]
    w16_pool, ina_pool = env["w16_pool"], env["ina_pool"]
    prod_pool, psum_pool = env["prod_pool"], env["psum_pool"]
    wst_pool, out_pool = env["wst_pool"], env["out_pool"]

    wt16 = w16_pool.tile([128, WCOLS], f16, tag="wt16")

    def load_weights():
        wview = wt16.rearrange("p (k y xx) -> p k y xx", k=9, xx=W)
        for kb in range(3):
            if "nodma" not in MODE:
                ws = wst_pool.tile([128, WCH], f32, tag="ws")
                nc.sync.dma_start(ws[:], wgt[kb])
            for dk in range(3):
                k = kb * 3 + dk
                di, dj = TAPS[k]
                if "nodma" not in MODE:
                    # per-tap conversion: the first product only needs tap 0
                    nc.scalar.copy(wt16[:, k * GOUT:(k + 1) * GOUT],
                                   ws[:, dk * GOUT:(dk + 1) * GOUT])
                if dj != 0:
                    col = 0 if dj == -1 else W - 1
                    nc.gpsimd.memset(wview[:, k, :, col:col + 1], 0.0)

    stage = {}

    def input_stage(p):
        ita = ina_pool.tile([128, PBLK], f16, tag="ita")
        # fp32->fp16 casting DMA; halos and zero pads are host-baked.
        # pair 0 is split per slot so the first products can start as soon
        # as the first half lands (region-level deps)
        if "nodma" not in MODE:
            if p == 0:
                nc.gpsimd.dma_start(ita[:, 0:GBLK], x[p, :, 0:GBLK])
                nc.gpsimd.dma_start(ita[:, GBLK:PBLK], x[p, :, GBLK:PBLK])
            else:
                nc.gpsimd.dma_start(ita[:], x[p])
        stage[p] = ita

    def compute_stage(p):
        ita = stage.pop(p)
        ot = out_pool.tile([128, 2 * GOUT], f32, tag="ot")
        for slot in range(2):
            boff = slot * GBLK
            ps = psum_pool.tile([128, GOUT], f32, tag="ps")
            for k, (di, dj) in enumerate(TAPS):
                pk = prod_pool.tile([128, GOUT], f16, tag="pk")
                s = boff + 2 + (1 + di) * W + dj
                nc.vector.tensor_mul(pk[:],
                                     wt16[:, k * GOUT:(k + 1) * GOUT],
                                     ita[:, s:s + GOUT])
                for c in range(0, GOUT, MM_N):
                    nc.tensor.matmul(ps[:, c:c + MM_N], ident[:],
                                     pk[:, c:c + MM_N],
                                     start=(k == 0), stop=(k == 8))
            nc.scalar.copy(ot[:, slot * GOUT:(slot + 1) * GOUT], ps[:])
        if "nodma" not in MODE:
            nc.scalar.dma_start(o[p], ot[:])

    input_stage(0)
    load_weights()
    for p in range(1, PIPE):
        input_stage(p)
    for p in range(8):
        if p + PIPE < 8:
            input_stage(p + PIPE)
        compute_stage(p)


def build_program(reps=1):
    nc = bacc.Bacc("TRN2", target_bir_lowering=False, debug=False,
                   enable_asserts=True, num_devices=N_CORES)
    f32 = mybir.dt.float32
    x = nc.dram_tensor("x", [8, 128, PBLK], f32, kind="ExternalInput").ap()
    wgt = nc.dram_tensor("w", [3, 128, WCH], f32, kind="ExternalInput").ap()
    o = nc.dram_tensor("o", [8, 128, 2 * GOUT], f32,
                       kind="ExternalOutput").ap()
    with tile.TileContext(nc) as tc:
        emit_kernel(tc, x, wgt, o, reps=reps)
    nc.compile()
    return nc


_CACHED_NC = None


def _get_nc():
    global _CACHED_NC
    if _CACHED_NC is None:
        _CACHED_NC = build_program()
    return _CACHED_NC


def run(inputs, trace=False):
    inp = np.ascontiguousarray(np.asarray(inputs["input"], dtype=np.float32))
    wgt = np.ascontiguousarray(np.asarray(inputs["weight"], dtype=np.float32))
    assert inp.shape == (N_TOTAL, C_X, H, W)
    assert wgt.shape == (N_TOTAL, C_W, 9, H * W)

    nc = _get_nc()
    inp5 = inp.reshape(N_TOTAL, G16, C_W, H, W)
    in_maps = []
    for c in range(N_CORES):
        bq, yh = divmod(c, 2)
        nsl = slice(4 * bq, 4 * bq + 4)
        # input blocks: [g, n, cw, 2180] with halo rows & zeros baked in
        blk = np.zeros((G16, N4, C_W, GBLK), np.float32)
        r0 = 32 * yh - 1
        lo, hi = max(r0, 0), min(r0 + ROWS, H)
        pad = lo - r0
        blk[..., 2 + pad * W:2 + (pad + hi - lo) * W] = \
            inp5[nsl, :, :, lo:hi].transpose(1, 0, 2, 3, 4).reshape(
                G16, N4, C_W, (hi - lo) * W)
        xc = blk.reshape(8, 2, N4, C_W, GBLK).transpose(0, 2, 3, 1, 4)
        # weights: y-half slice, 3-tap chunks [3, 128, 6144]
        wh = wgt[nsl].reshape(N4, C_W, 9, H * W)[
            :, :, :, yh * GOUT:(yh + 1) * GOUT]
        wc = wh.transpose(2, 0, 1, 3).reshape(3, 3, 128, GOUT) \
            .transpose(0, 2, 1, 3)
        in_maps.append({
            "x": np.ascontiguousarray(xc).reshape(8, 128, PBLK),
            "w": np.ascontiguousarray(wc).reshape(3, 128, WCH),
        })
    res = run_bass_kernel_spmd(nc, in_maps, core_ids=list(range(N_CORES)),
                               trace=trace)
    out = np.empty((N_TOTAL, C_X, H * W), dtype=np.float32)
    for c in range(N_CORES):
        bq, yh = divmod(c, 2)
        oc = res.results[c]["o"].reshape(8, N4, C_W, 2, GOUT) \
            .transpose(1, 0, 3, 2, 4).reshape(N4, C_X, GOUT)
        out[4 * bq:4 * bq + 4, :, yh * GOUT:(yh + 1) * GOUT] = oc
    return out.reshape(N_TOTAL, C_X, H, W), res


def kernel(**inputs):
    out, _ = run(inputs)
    return out


# revision 8
# speedup vs baseline: 1.1765x; 1.0361x over previous
"""Involution-style aggregation — v4: spatial-half sharding (see kernel.py v3).

Core = (batch-quad bq, y-half yh): batches 4bq..4bq+4, output rows
32yh..32yh+32, ALL 512 channels (16 groups). vs v3's channel-half sharding
this halves per-core weight traffic (9.4MB vs 18.9MB — each weight element
is read once fleet-wide) and shrinks the body-start weight ramp.

Host bakes halo rows and zero padding into the permuted input (blocks of
[z2, 34 rows x 64, z2] = 2180 fp16 elems, two groups packed per 17.4KB-desc
casting DMA), so the device does no pad memsets at all. Weights stream as
3-tap fp32 chunks on SP + ACT conversion into a resident [128, 18432] fp16
tile. Products all on DVE (2x fp16, unaligned APs fine); PE identity-matmul
tap accumulation into PSUM; ACT evacuates; 2-group 16KB-desc stores on SP.
"""

import numpy as np

import concourse.bacc as bacc
import concourse.mybir as mybir
import concourse.tile as tile
from concourse.bass_utils import run_bass_kernel_spmd

# Problem constants (hardcoded per harness contract)
N_TOTAL, C_X, H, W = 16, 512, 64, 64
C_W = 32
N_CORES = 8
N4 = 4              # batches per core
G16 = 16            # groups per core
ROWS = 34           # 32 output rows + 2 halo rows per block
GBLK = 2 + ROWS * W + 2   # 2180: [z2, 34 rows, z2]
PBLK = 2 * GBLK     # 4360: two groups per input DMA
GOUT = 32 * W       # 2048 output elems per group
WCOLS = 9 * GOUT    # 18432 weight elems per partition
WCH = 3 * GOUT      # 6144: 3-tap weight chunk
MM_N = 512
PIPE = 4
MODE = {"full"}            # pair lookahead

TAPS = [(di, dj) for di in (-1, 0, 1) for dj in (-1, 0, 1)]


def emit_kernel(tc, x, wgt, o, reps=1):
    nc = tc.nc
    f32 = mybir.dt.float32
    f16 = mybir.dt.float16

    ident_dram = nc.inline_tensor(np.eye(128, dtype=np.float16), name="ident")

    with (
        tc.tile_pool(name="const", bufs=1) as const_pool,
        tc.tile_pool(name="w16", bufs=1) as w16_pool,
        tc.tile_pool(name="ina", bufs=5) as ina_pool,
        tc.tile_pool(name="prod", bufs=8) as prod_pool,
        tc.tile_pool(name="psum", bufs=2, space="PSUM") as psum_pool,
        tc.tile_pool(name="wst", bufs=2) as wst_pool,
        tc.tile_pool(name="outp", bufs=2) as out_pool,
    ):
        ident = const_pool.tile([128, 128], f16)
        nc.sync.dma_start(ident[:], ident_dram.ap())
        env = dict(locals())
        if reps == 1:
            _emit_body(tc, env)
        else:
            with tc.For_i(0, reps, 1):
                _emit_body(tc, env)


def _emit_body(tc, env):
    nc = env["nc"]
    f32, f16 = env["f32"], env["f16"]
    x, wgt, o, ident = env["x"], env["wgt"], env["o"], env["ident"]
    w16_pool, ina_pool = env["w16_pool"], env["ina_pool"]
    prod_pool, psum_pool = env["prod_pool"], env["psum_pool"]
    wst_pool, out_pool = env["wst_pool"], env["out_pool"]

    wt16 = w16_pool.tile([128, WCOLS], f16, tag="wt16")

    def load_weights():
        wview = wt16.rearrange("p (k y xx) -> p k y xx", k=9, xx=W)
        for kb in range(3):
            if "nodma" not in MODE:
                ws = wst_pool.tile([128, WCH], f32, tag="ws")
                nc.sync.dma_start(ws[:], wgt[kb])
            for dk in range(3):
                k = kb * 3 + dk
                di, dj = TAPS[k]
                if "nodma" not in MODE:
                    # per-tap conversion: the first product only needs tap 0
                    nc.scalar.copy(wt16[:, k * GOUT:(k + 1) * GOUT],
                                   ws[:, dk * GOUT:(dk + 1) * GOUT])
                if dj != 0:
                    col = 0 if dj == -1 else W - 1
                    nc.gpsimd.memset(wview[:, k, :, col:col + 1], 0.0)

    stage = {}

    def input_stage(p):
        ita = ina_pool.tile([128, PBLK], f16, tag="ita")
        # fp32->fp16 casting DMA; halos and zero pads are host-baked.
        # pair 0 is split per slot so the first products can start as soon
        # as the first half lands (region-level deps)
        if "nodma" not in MODE:
            if p == 0:
                nc.gpsimd.dma_start(ita[:, 0:GBLK], x[p, :, 0:GBLK])
                nc.gpsimd.dma_start(ita[:, GBLK:PBLK], x[p, :, GBLK:PBLK])
            else:
                nc.gpsimd.dma_start(ita[:], x[p])
        stage[p] = ita

    def compute_stage(p):
        ita = stage.pop(p)
        ot = out_pool.tile([128, 2 * GOUT], f32, tag="ot")
        for slot in range(2):
            boff = slot * GBLK
            ps = psum_pool.tile([128, GOUT], f32, tag="ps")
            for k, (di, dj) in enumerate(TAPS):
                pk = prod_pool.tile([128, GOUT], f16, tag="pk")
                s = boff + 2 + (1 + di) * W + dj
                nc.vector.tensor_mul(pk[:],
                                     wt16[:, k * GOUT:(k + 1) * GOUT],
                                     ita[:, s:s + GOUT])
                for c in range(0, GOUT, MM_N):
                    nc.tensor.matmul(ps[:, c:c + MM_N], ident[:],
                                     pk[:, c:c + MM_N],
                                     start=(k == 0), stop=(k == 8))
            nc.scalar.copy(ot[:, slot * GOUT:(slot + 1) * GOUT], ps[:])
        if "nodma" not in MODE:
            nc.sync.dma_start(o[p], ot[:])

    input_stage(0)
    load_weights()
    for p in range(1, PIPE):
        input_stage(p)
    for p in range(8):
        if p + PIPE < 8:
            input_stage(p + PIPE)
        compute_stage(p)


def build_program(reps=1):
    nc = bacc.Bacc("TRN2", target_bir_lowering=False, debug=False,
                   enable_asserts=True, num_devices=N_CORES)
    f32 = mybir.dt.float32
    x = nc.dram_tensor("x", [8, 128, PBLK], f32, kind="ExternalInput").ap()
    wgt = nc.dram_tensor("w", [3, 128, WCH], f32, kind="ExternalInput").ap()
    o = nc.dram_tensor("o", [8, 128, 2 * GOUT], f32,
                       kind="ExternalOutput").ap()
    with tile.TileContext(nc) as tc:
        emit_kernel(tc, x, wgt, o, reps=reps)
    nc.compile()
    return nc


_CACHED_NC = None


def _get_nc():
    global _CACHED_NC
    if _CACHED_NC is None:
        _CACHED_NC = build_program()
    return _CACHED_NC


def run(inputs, trace=False):
    inp = np.ascontiguousarray(np.asarray(inputs["input"], dtype=np.float32))
    wgt = np.ascontiguousarray(np.asarray(inputs["weight"], dtype=np.float32))
    assert inp.shape == (N_TOTAL, C_X, H, W)
    assert wgt.shape == (N_TOTAL, C_W, 9, H * W)

    nc = _get_nc()
    inp5 = inp.reshape(N_TOTAL, G16, C_W, H, W)
    in_maps = []
    for c in range(N_CORES):
        bq, yh = divmod(c, 2)
        nsl = slice(4 * bq, 4 * bq + 4)
        # input blocks: [g, n, cw, 2180] with halo rows & zeros baked in
        blk = np.zeros((G16, N4, C_W, GBLK), np.float32)
        r0 = 32 * yh - 1
        lo, hi = max(r0, 0), min(r0 + ROWS, H)
        pad = lo - r0
        blk[..., 2 + pad * W:2 + (pad + hi - lo) * W] = \
            inp5[nsl, :, :, lo:hi].transpose(1, 0, 2, 3, 4).reshape(
                G16, N4, C_W, (hi - lo) * W)
        xc = blk.reshape(8, 2, N4, C_W, GBLK).transpose(0, 2, 3, 1, 4)
        # weights: y-half slice, 3-tap chunks [3, 128, 6144]
        wh = wgt[nsl].reshape(N4, C_W, 9, H * W)[
            :, :, :, yh * GOUT:(yh + 1) * GOUT]
        wc = wh.transpose(2, 0, 1, 3).reshape(3, 3, 128, GOUT) \
            .transpose(0, 2, 1, 3)
        in_maps.append({
            "x": np.ascontiguousarray(xc).reshape(8, 128, PBLK),
            "w": np.ascontiguousarray(wc).reshape(3, 128, WCH),
        })
    res = run_bass_kernel_spmd(nc, in_maps, core_ids=list(range(N_CORES)),
                               trace=trace)
    out = np.empty((N_TOTAL, C_X, H * W), dtype=np.float32)
    for c in range(N_CORES):
        bq, yh = divmod(c, 2)
        oc = res.results[c]["o"].reshape(8, N4, C_W, 2, GOUT) \
            .transpose(1, 0, 3, 2, 4).reshape(N4, C_X, GOUT)
        out[4 * bq:4 * bq + 4, :, yh * GOUT:(yh + 1) * GOUT] = oc
    return out.reshape(N_TOTAL, C_X, H, W), res


def kernel(**inputs):
    out, _ = run(inputs)
    return out
